# revision 10
# baseline (speedup 1.0000x reference)
"""Trainium2 Bass kernel for batch-axis-softmax attention.

Reference computation (B=8, S=2048, D_IN=512, D_OUT=256):
    q = relu(x @ Wq + bq); k = relu(x @ Wk + bk); v = relu(x @ Wv + bv)
    scores = q @ k^T / sqrt(256)            # [B, S, S]
    attn = softmax(scores, axis=0)          # softmax over the BATCH axis
    out = attn @ v                          # [B, S, D_OUT]

Sharding (8 cores): query-axis sharding. Core c owns query rows
[c*256, (c+1)*256) of every batch. Because the softmax runs over the batch
axis, each (q, k) position needs all 8 batches' scores — by giving every
core all batches for a slice of q-rows, the softmax is core-local.

Per core:
  phase 1: compute k^T (fp32) and v (bf16) for its own batch b=c from a
           host-pre-transposed x[c]^T, write to a DRAM bounce buffer.
  AllGather (one collective, 3 MB/rank payload) -> every core has k/v of
           all 8 batches.
  phase 2: project q for its 256 q-rows of all batches (from host-sliced
           x), scores^T = k_b @ q_slice^T, exp (no max needed: scores in
           [0.18, 2.2]), Z = sum_b exp, attn = exp/Z, out = attn^T @ v.

Matmuls use float32r (full PE rate at N>=256, near-fp32 accuracy).
exp/attn are stored bf16 (SBUF budget); the k-sum in attn @ v averages the
elementwise bf16 noise down by ~sqrt(2048).
"""

import numpy as np

import concourse.bacc as bacc
import concourse.mybir as mybir
import concourse.tile as tile
from concourse import bass_utils

F32 = mybir.dt.float32
F32R = mybir.dt.float32r
BF16 = mybir.dt.bfloat16

B = 8            # batch == number of cores
S = 2048         # sequence length
D = 512          # input dim
E = 256          # output dim
P = 128          # partitions
N_CORES = 8
QS = S // N_CORES  # q rows per core = 256

DC = D // P      # 4 d-chunks
EC = E // P      # 2 e-chunks
SC = S // P      # 16 s-chunks
SCALE = 1.0 / 16.0  # 1/sqrt(E)

# cc payload rows: k^T is [E, S] f32 (rows 0..255), v is [S, E] bf16 packed
# as [128, S*E/128] -> 128 f32-viewed rows (rows 256..383)
CC_ROWS = E + P  # 384


def build_nc():
    nc = bacc.Bacc("TRN2", target_bir_lowering=False, debug=False,
                   num_devices=N_CORES)

    xt_d = nc.dram_tensor("xt", [D, S], F32, kind="ExternalInput")
    xq_d = nc.dram_tensor("xq", [B, D, QS], F32, kind="ExternalInput")
    wq_d = nc.dram_tensor("wq", [D, E], F32, kind="ExternalInput")
    wk_d = nc.dram_tensor("wk", [D, E], F32, kind="ExternalInput")
    wv_d = nc.dram_tensor("wv", [D, E], F32, kind="ExternalInput")
    bq_d = nc.dram_tensor("bq", [E], F32, kind="ExternalInput")
    bk_d = nc.dram_tensor("bk", [E], F32, kind="ExternalInput")
    bv_d = nc.dram_tensor("bv", [E], F32, kind="ExternalInput")
    ones_d = nc.dram_tensor("onesv", [1, P], F32, kind="ExternalInput")
    out_d = nc.dram_tensor("out", [B, QS, E], F32, kind="ExternalOutput")

    def mm(out, lhsT, rhs, start, stop):
        nc.tensor.matmul(out, lhsT, rhs, start=start, stop=stop)

    with tile.TileContext(nc) as tc:
        with tc.tile_pool(name="const", bufs=1) as cpool, \
             tc.tile_pool(name="dram", bufs=1, space="DRAM") as dram, \
             tc.tile_pool(name="stream", bufs=2) as stream, \
             tc.tile_pool(name="outp", bufs=4) as outpool:

            cc_in = dram.tile([CC_ROWS, S], F32)
            cc_out = dram.tile([N_CORES * CC_ROWS, S], F32,
                               addr_space="Shared")

            # ---- constants ----
            wq_sb = cpool.tile([P, DC, E], F32R)
            wk_sb = cpool.tile([P, DC, E], F32R)
            wv_sb = cpool.tile([P, DC, E], F32R)
            nc.sync.dma_start(wq_sb[:], wq_d.ap().rearrange(
                "(dc p) e -> p dc e", p=P).bitcast(F32R))
            nc.sync.dma_start(wk_sb[:], wk_d.ap().rearrange(
                "(dc p) e -> p dc e", p=P).bitcast(F32R))
            nc.sync.dma_start(wv_sb[:], wv_d.ap().rearrange(
                "(dc p) e -> p dc e", p=P).bitcast(F32R))
            bq_sb = cpool.tile([P, EC], F32)
            bk_sb = cpool.tile([P, EC], F32)
            nc.sync.dma_start(bq_sb[:], bq_d.ap().rearrange(
                "(ec p) -> p ec", p=P))
            nc.sync.dma_start(bk_sb[:], bk_d.ap().rearrange(
                "(ec p) -> p ec", p=P))
            bv_row = cpool.tile([1, E], F32R)
            nc.sync.dma_start(bv_row[:], bv_d.ap().rearrange(
                "(a e) -> a e", a=1).bitcast(F32R))
            ones_row = cpool.tile([1, P], F32R)
            nc.sync.dma_start(ones_row[:], ones_d.ap().bitcast(F32R))

            # ---- phase 1: k^T and v for own batch ----
            with tc.tile_pool(name="p1", bufs=1) as p1pool, \
                 tc.tile_pool(name="p1ps", bufs=1, space="PSUM") as p1ps:
                xt_sb = p1pool.tile([P, DC, S], F32R)
                nc.sync.dma_start(xt_sb[:], xt_d.ap().rearrange(
                    "(dc p) s -> p dc s", p=P).bitcast(F32R))

                # k^T[e, s] = relu(Wk^T @ x^T + bk): lhsT = Wk chunk,
                # rhs = x^T chunk
                kt_sb = p1pool.tile([P, EC, S], F32)
                for ec in range(EC):
                    for sh in range(2):  # halves of S
                        ps_k = p1ps.tile([P, 1024], F32, tag="kps", bufs=2)
                        for dc in range(DC):
                            for st in range(2):  # 512-wide moving tiles
                                mm(ps_k[:, st * 512:(st + 1) * 512],
                                   wk_sb[:, dc, ec * P:(ec + 1) * P],
                                   xt_sb[:, dc,
                                         sh * 1024 + st * 512:
                                         sh * 1024 + (st + 1) * 512],
                                   start=(dc == 0), stop=(dc == DC - 1))
                        nc.scalar.activation(
                            kt_sb[:, ec, sh * 1024:(sh + 1) * 1024],
                            ps_k[:],
                            mybir.ActivationFunctionType.Relu,
                            bias=bk_sb[:, ec:ec + 1])
                nc.sync.dma_start(
                    cc_in[0:E, :].rearrange("(ec p) s -> p ec s", p=P),
                    kt_sb[:])

                # v[s, e] = relu(x @ Wv + bv): lhsT = x^T chunk (stationary),
                # rhs = Wv chunk; bias via rank-1 matmul ones^T @ bv_row
                v_sb = p1pool.tile([P, SC * E], BF16)
                for sp in range(SC // 2):
                    ps_v = p1ps.tile([P, 2 * E], F32, tag="vps", bufs=2)
                    for half in range(2):
                        st = sp * 2 + half
                        sl = ps_v[:, half * E:(half + 1) * E]
                        mm(sl, ones_row[0:1, :], bv_row[0:1, :],
                           start=True, stop=False)
                        for dc in range(DC):
                            mm(sl, xt_sb[:, dc, st * P:(st + 1) * P],
                               wv_sb[:, dc, :],
                               start=False, stop=(dc == DC - 1))
                    nc.scalar.activation(
                        v_sb[:, sp * 2 * E:(sp + 1) * 2 * E], ps_v[:],
                        mybir.ActivationFunctionType.Relu)
                nc.sync.dma_start(cc_in[E:CC_ROWS, :].bitcast(BF16), v_sb[:])

            # ---- the one collective ----
            nc.gpsimd.collective_compute(
                "AllGather",
                mybir.AluOpType.bypass,
                replica_groups=[list(range(N_CORES))],
                ins=[cc_in[:]],
                outs=[cc_out[:]],
            )

            # ---- phase 2a: q projection for own q-slice, all batches ----
            p2cm = tc.tile_pool(name="p2", bufs=1)
            p2pool = p2cm.__enter__()
            qsl_sb = p2pool.tile([P, B, EC, QS], F32R)
            with tc.tile_pool(name="p2q", bufs=1) as qpool, \
                 tc.tile_pool(name="p2qps", bufs=1, space="PSUM") as qps:
                xq_sb = qpool.tile([P, B, DC, QS], F32R)
                nc.sync.dma_start(xq_sb[:], xq_d.ap().rearrange(
                    "b (dc p) s -> p b dc s", p=P).bitcast(F32R))
                for ec in range(EC):
                    ps_list = []
                    for b in range(B):
                        ps_q = qps.tile([P, QS], F32, tag="qps", bufs=B,
                                        name=f"ps_q_{ec}_{b}")
                        ps_list.append(ps_q)
                    for dc in range(DC):
                        for b in range(B):
                            mm(ps_list[b][:],
                               wq_sb[:, dc, ec * P:(ec + 1) * P],
                               xq_sb[:, b, dc, :],
                               start=(dc == 0), stop=(dc == DC - 1))
                    for b in range(B):
                        nc.scalar.activation(
                            qsl_sb[:, b, ec, :], ps_list[b][:],
                            mybir.ActivationFunctionType.Relu,
                            bias=bq_sb[:, ec:ec + 1])

            # ---- phase 2b: scores^T, exp, Z ----
            # exp_all[p, b, kc, q] = exp(scores[b, q, kc*128+p] / 16)
            exp_all = p2pool.tile([P, B, SC, QS], BF16)
            z_sb = p2pool.tile([P, SC, QS], F32)
            r_sb = p2pool.tile([P, SC, QS], F32)
            with tc.tile_pool(name="p2sps", bufs=1, space="PSUM") as sps:
                for b in range(B):
                    kt_s = stream.tile([P, EC, S], F32R, tag="kv",
                                       name=f"kt_s{b}")
                    nc.sync.dma_start(
                        kt_s[:],
                        cc_out[b * CC_ROWS:b * CC_ROWS + E, :].rearrange(
                            "(ec p) s -> p ec s", p=P).bitcast(F32R))
                    for half in range(2):
                        ps_s = sps.tile([P, 8, QS], F32, tag="sps", bufs=2,
                                        name=f"ps_s{b}_{half}")
                        for kc8 in range(8):
                            kc = half * 8 + kc8
                            for ec in range(EC):
                                mm(ps_s[:, kc8, :],
                                   kt_s[:, ec, kc * P:(kc + 1) * P],
                                   qsl_sb[:, b, ec, :],
                                   start=(ec == 0), stop=(ec == EC - 1))
                        nc.scalar.activation(
                            exp_all[:, b, half * 8:(half + 1) * 8, :],
                            ps_s[:],
                            mybir.ActivationFunctionType.Exp,
                            scale=SCALE)
                    if b == 0:
                        nc.vector.tensor_copy(z_sb[:], exp_all[:, 0])
                    else:
                        nc.vector.tensor_add(z_sb[:], z_sb[:],
                                             exp_all[:, b])

            # ---- phase 2c: attn = exp * (1/Z), out = attn^T @ v ----
            nc.vector.reciprocal(r_sb[:], z_sb[:])
            for b in range(B):
                nc.vector.tensor_mul(exp_all[:, b], exp_all[:, b], r_sb[:])

            with tc.tile_pool(name="p2ops", bufs=1, space="PSUM") as ops:
                for b in range(B):
                    v_s = stream.tile([P, SC * E], BF16, tag="kv",
                                      name=f"v_s{b}")
                    nc.sync.dma_start(
                        v_s[:],
                        cc_out[b * CC_ROWS + E:
                               (b + 1) * CC_ROWS, :].bitcast(BF16))
                    for qc in range(2):
                        ps_o = ops.tile([P, E], F32, tag="ops", bufs=4,
                                        name=f"ps_o{b}_{qc}")
                        for st in range(SC):
                            nc.tensor.matmul(
                                ps_o[:],
                                exp_all[:, b, st, qc * P:(qc + 1) * P],
                                v_s[:, st * E:(st + 1) * E],
                                start=(st == 0), stop=(st == SC - 1))
                        o_sb = outpool.tile([P, E], F32, tag="osb",
                                            name=f"o_sb{b}_{qc}")
                        nc.vector.tensor_copy(o_sb[:], ps_o[:])
                        nc.sync.dma_start(
                            out_d.ap()[b, qc * P:(qc + 1) * P, :], o_sb[:])
            p2cm.__exit__(None, None, None)

    nc.compile()
    return nc


_NC_CACHE = []


def _get_nc():
    if not _NC_CACHE:
        _NC_CACHE.append(build_nc())
    return _NC_CACHE[0]


def make_in_maps(x, Wq, bq, Wk, bk, Wv, bv):
    in_maps = []
    for c in range(N_CORES):
        xt = np.ascontiguousarray(x[c].T)  # [D, S]
        xq = np.ascontiguousarray(
            x[:, c * QS:(c + 1) * QS, :].transpose(0, 2, 1))  # [B, D, QS]
        in_maps.append({
            "xt": xt, "xq": xq,
            "wq": Wq, "wk": Wk, "wv": Wv,
            "bq": bq, "bk": bk, "bv": bv,
            "onesv": np.ones((1, P), np.float32),
        })
    return in_maps


def kernel(x, Wq, bq, Wk, bk, Wv, bv):
    x = np.asarray(x, dtype=np.float32)
    Wq = np.asarray(Wq, dtype=np.float32)
    Wk = np.asarray(Wk, dtype=np.float32)
    Wv = np.asarray(Wv, dtype=np.float32)
    bq = np.asarray(bq, dtype=np.float32)
    bk = np.asarray(bk, dtype=np.float32)
    bv = np.asarray(bv, dtype=np.float32)

    nc = _get_nc()
    in_maps = make_in_maps(x, Wq, bq, Wk, bk, Wv, bv)
    res = bass_utils.run_bass_kernel_spmd(
        nc, in_maps, core_ids=list(range(N_CORES)))
    out = np.empty((B, S, E), np.float32)
    for c in range(N_CORES):
        out[:, c * QS:(c + 1) * QS, :] = res.results[c]["out"]
    return out


# revision 12
# speedup vs baseline: 1.7135x; 1.7135x over previous
"""Trainium2 Bass kernel for batch-axis-softmax attention.

Reference computation (B=8, S=2048, D_IN=512, D_OUT=256):
    q = relu(x @ Wq + bq); k = relu(x @ Wk + bk); v = relu(x @ Wv + bv)
    scores = q @ k^T / sqrt(256)            # [B, S, S]
    attn = softmax(scores, axis=0)          # softmax over the BATCH axis
    out = attn @ v                          # [B, S, D_OUT]

Sharding (8 cores): query-axis sharding. Core c owns query rows
[c*256, (c+1)*256) of every batch. Because the softmax runs over the batch
axis, each (q, k) position needs all 8 batches' scores — by giving every
core all batches for a slice of q-rows, the softmax is core-local.

Per core:
  phase 1: compute k^T and v (both bf16) for its own batch b=c from a
           host-pre-transposed x[c]^T.
  Two AllGathers (k first so the scores stage can start while v is still
  in flight), 1 MB/rank payload each.
  phase 2: project q for its 256 q-rows of all batches (from host-sliced
           x), scores^T = k_b @ q_slice^T (bf16 in, f32 accum), exp (no
           max subtraction needed: scores are in [0.18, 2.2]), Z = sum_b
           exp, attn = exp/Z, out = attn^T @ v.

Projections run in float32r (full PE rate at N>=256, near-fp32 accuracy);
the attention matmuls run bf16 (FWL weight loads). exp/attn/Z are bf16 —
the k-sum in attn @ v averages elementwise bf16 noise down by ~sqrt(2048).
"""

import numpy as np

import concourse.bacc as bacc
import concourse.mybir as mybir
import concourse.tile as tile
from concourse import bass_utils

F32 = mybir.dt.float32
F32R = mybir.dt.float32r
BF16 = mybir.dt.bfloat16

B = 8            # batch == number of cores
S = 2048         # sequence length
D = 512          # input dim
E = 256          # output dim
P = 128          # partitions
N_CORES = 8
QS = S // N_CORES  # q rows per core = 256

DC = D // P      # 4 d-chunks
EC = E // P      # 2 e-chunks
SC = S // P      # 16 s-chunks
SCALE = 1.0 / 16.0  # 1/sqrt(E)


def build_nc():
    nc = bacc.Bacc("TRN2", target_bir_lowering=False, debug=False,
                   num_devices=N_CORES)

    xt_d = nc.dram_tensor("xt", [D, S], F32, kind="ExternalInput")
    xq_d = nc.dram_tensor("xq", [B, D, QS], F32, kind="ExternalInput")
    wq_d = nc.dram_tensor("wq", [D, E], F32, kind="ExternalInput")
    wk_d = nc.dram_tensor("wk", [D, E], F32, kind="ExternalInput")
    wv_d = nc.dram_tensor("wv", [D, E], F32, kind="ExternalInput")
    bq_d = nc.dram_tensor("bq", [E], F32, kind="ExternalInput")
    bk_d = nc.dram_tensor("bk", [E], F32, kind="ExternalInput")
    bv_d = nc.dram_tensor("bv", [E], F32, kind="ExternalInput")
    ones_d = nc.dram_tensor("onesv", [1, P], F32, kind="ExternalInput")
    out_d = nc.dram_tensor("out", [B, QS, E], F32, kind="ExternalOutput")

    def mm(out, lhsT, rhs, start, stop):
        nc.tensor.matmul(out, lhsT, rhs, start=start, stop=stop)

    with tile.TileContext(nc) as tc:
        with tc.tile_pool(name="const", bufs=1) as cpool, \
             tc.tile_pool(name="dram", bufs=1, space="DRAM") as dram, \
             tc.tile_pool(name="stream", bufs=3) as stream, \
             tc.tile_pool(name="outp", bufs=4) as outpool:

            # k^T / v of own batch, bf16, flattened [128, *] viewed as f32
            cc_k = dram.tile([P, S], F32)
            cc_v = dram.tile([P, S], F32)
            cc_k_out = dram.tile([N_CORES * P, S], F32, addr_space="Shared")
            cc_v_out = dram.tile([N_CORES * P, S], F32, addr_space="Shared")

            # ---- constants ----
            wq_sb = cpool.tile([P, DC, E], F32R)
            wk_sb = cpool.tile([P, DC, E], F32R)
            wv_sb = cpool.tile([P, DC, E], F32R)
            nc.sync.dma_start(wq_sb[:], wq_d.ap().rearrange(
                "(dc p) e -> p dc e", p=P).bitcast(F32R))
            nc.sync.dma_start(wk_sb[:], wk_d.ap().rearrange(
                "(dc p) e -> p dc e", p=P).bitcast(F32R))
            nc.sync.dma_start(wv_sb[:], wv_d.ap().rearrange(
                "(dc p) e -> p dc e", p=P).bitcast(F32R))
            bq_sb = cpool.tile([P, EC], F32)
            bk_sb = cpool.tile([P, EC], F32)
            nc.sync.dma_start(bq_sb[:], bq_d.ap().rearrange(
                "(ec p) -> p ec", p=P))
            nc.sync.dma_start(bk_sb[:], bk_d.ap().rearrange(
                "(ec p) -> p ec", p=P))
            bv_row = cpool.tile([1, E], F32R)
            nc.sync.dma_start(bv_row[:], bv_d.ap().rearrange(
                "(a e) -> a e", a=1).bitcast(F32R))
            ones_row = cpool.tile([1, P], F32R)
            nc.sync.dma_start(ones_row[:], ones_d.ap().bitcast(F32R))

            # ---- phase 1: k^T and v for own batch ----
            with tc.tile_pool(name="p1", bufs=1) as p1pool, \
                 tc.tile_pool(name="p1ps", bufs=1, space="PSUM") as p1ps:
                xt_sb = p1pool.tile([P, DC, S], F32R)
                nc.sync.dma_start(xt_sb[:], xt_d.ap().rearrange(
                    "(dc p) s -> p dc s", p=P).bitcast(F32R))

                # k^T[e, s] = relu(Wk^T @ x^T + bk): lhsT = Wk chunk,
                # rhs = x^T chunk.  kt layout: [p, ec, s] bf16
                kt_sb = p1pool.tile([P, EC, S], BF16)
                for ec in range(EC):
                    for sh in range(2):  # halves of S
                        ps_k = p1ps.tile([P, 1024], F32, tag="kps", bufs=2)
                        for dc in range(DC):
                            for st in range(2):  # 512-wide moving tiles
                                mm(ps_k[:, st * 512:(st + 1) * 512],
                                   wk_sb[:, dc, ec * P:(ec + 1) * P],
                                   xt_sb[:, dc,
                                         sh * 1024 + st * 512:
                                         sh * 1024 + (st + 1) * 512],
                                   start=(dc == 0), stop=(dc == DC - 1))
                        nc.scalar.activation(
                            kt_sb[:, ec, sh * 1024:(sh + 1) * 1024],
                            ps_k[:],
                            mybir.ActivationFunctionType.Relu,
                            bias=bk_sb[:, ec:ec + 1])
                nc.sync.dma_start(cc_k[:].bitcast(BF16), kt_sb[:])

                # gather k early: scores can start as soon as this lands
                nc.gpsimd.collective_compute(
                    "AllGather",
                    mybir.AluOpType.bypass,
                    replica_groups=[list(range(N_CORES))],
                    ins=[cc_k[:]],
                    outs=[cc_k_out[:]],
                )

                # v[s, e] = relu(x @ Wv + bv): lhsT = x^T chunk (stationary),
                # rhs = Wv chunk; bias via rank-1 matmul ones^T @ bv_row
                v_sb = p1pool.tile([P, SC * E], BF16)
                for sp in range(SC // 2):
                    ps_v = p1ps.tile([P, 2 * E], F32, tag="vps", bufs=2)
                    for half in range(2):
                        st = sp * 2 + half
                        sl = ps_v[:, half * E:(half + 1) * E]
                        mm(sl, ones_row[0:1, :], bv_row[0:1, :],
                           start=True, stop=False)
                        for dc in range(DC):
                            mm(sl, xt_sb[:, dc, st * P:(st + 1) * P],
                               wv_sb[:, dc, :],
                               start=False, stop=(dc == DC - 1))
                    nc.scalar.activation(
                        v_sb[:, sp * 2 * E:(sp + 1) * 2 * E], ps_v[:],
                        mybir.ActivationFunctionType.Relu)
                nc.sync.dma_start(cc_v[:].bitcast(BF16), v_sb[:])

                nc.gpsimd.collective_compute(
                    "AllGather",
                    mybir.AluOpType.bypass,
                    replica_groups=[list(range(N_CORES))],
                    ins=[cc_v[:]],
                    outs=[cc_v_out[:]],
                )

            # ---- phase 2a: q projection for own q-slice, all batches ----
            p2cm = tc.tile_pool(name="p2", bufs=1)
            p2pool = p2cm.__enter__()
            qsl_sb = p2pool.tile([P, B, EC, QS], BF16)
            with tc.tile_pool(name="p2q", bufs=1) as qpool, \
                 tc.tile_pool(name="p2qps", bufs=1, space="PSUM") as qps:
                xq_sb = qpool.tile([P, B, DC, QS], F32R)
                nc.sync.dma_start(xq_sb[:], xq_d.ap().rearrange(
                    "b (dc p) s -> p b dc s", p=P).bitcast(F32R))
                for ec in range(EC):
                    ps_list = []
                    for b in range(B):
                        ps_q = qps.tile([P, QS], F32, tag="qps", bufs=B,
                                        name=f"ps_q_{ec}_{b}")
                        ps_list.append(ps_q)
                    for dc in range(DC):
                        for b in range(B):
                            mm(ps_list[b][:],
                               wq_sb[:, dc, ec * P:(ec + 1) * P],
                               xq_sb[:, b, dc, :],
                               start=(dc == 0), stop=(dc == DC - 1))
                    for b in range(B):
                        nc.scalar.activation(
                            qsl_sb[:, b, ec, :], ps_list[b][:],
                            mybir.ActivationFunctionType.Relu,
                            bias=bq_sb[:, ec:ec + 1])

            # ---- phase 2b: scores^T, exp, Z ----
            # exp_all[p, b, kc, q] = exp(scores[b, q, kc*128+p] / 16)
            exp_all = p2pool.tile([P, B, SC, QS], BF16)
            z_sb = p2pool.tile([P, SC, QS], BF16)
            r_sb = p2pool.tile([P, SC, QS], BF16)
            with tc.tile_pool(name="p2sps", bufs=1, space="PSUM") as sps:
                for b in range(B):
                    kt_s = stream.tile([P, 2 * S], BF16, tag="kv",
                                       name=f"kt_s{b}")
                    nc.sync.dma_start(
                        kt_s[:],
                        cc_k_out[b * P:(b + 1) * P, :].bitcast(BF16))
                    for half in range(2):
                        ps_s = sps.tile([P, 8, QS], F32, tag="sps", bufs=2,
                                        name=f"ps_s{b}_{half}")
                        for kc8 in range(8):
                            kc = half * 8 + kc8
                            for ec in range(EC):
                                mm(ps_s[:, kc8, :],
                                   kt_s[:, ec * S + kc * P:
                                        ec * S + (kc + 1) * P],
                                   qsl_sb[:, b, ec, :],
                                   start=(ec == 0), stop=(ec == EC - 1))
                        nc.scalar.activation(
                            exp_all[:, b, half * 8:(half + 1) * 8, :],
                            ps_s[:],
                            mybir.ActivationFunctionType.Exp,
                            scale=SCALE)
                    if b == 0:
                        nc.vector.tensor_copy(z_sb[:], exp_all[:, 0])
                    else:
                        nc.vector.tensor_add(z_sb[:], z_sb[:],
                                             exp_all[:, b])

            # ---- phase 2c: attn = exp * (1/Z), out = attn^T @ v ----
            # bf16 1/Z noise is elementwise-independent across k positions;
            # the 2048-term k-sum in attn @ v averages it to ~1e-4.
            with nc.allow_low_precision(reason="1/Z noise washes in k-sum"):
                nc.vector.reciprocal(r_sb[:], z_sb[:])
            for b in range(B):
                nc.vector.tensor_mul(exp_all[:, b], exp_all[:, b], r_sb[:])

            with tc.tile_pool(name="p2ops", bufs=1, space="PSUM") as ops:
                for b in range(B):
                    v_s = stream.tile([P, SC * E], BF16, tag="kv",
                                      name=f"v_s{b}")
                    nc.sync.dma_start(
                        v_s[:],
                        cc_v_out[b * P:(b + 1) * P, :].bitcast(BF16))
                    for qc in range(2):
                        ps_o = ops.tile([P, E], F32, tag="ops", bufs=4,
                                        name=f"ps_o{b}_{qc}")
                        for st in range(SC):
                            nc.tensor.matmul(
                                ps_o[:],
                                exp_all[:, b, st, qc * P:(qc + 1) * P],
                                v_s[:, st * E:(st + 1) * E],
                                start=(st == 0), stop=(st == SC - 1))
                        o_sb = outpool.tile([P, E], F32, tag="osb",
                                            name=f"o_sb{b}_{qc}")
                        nc.vector.tensor_copy(o_sb[:], ps_o[:])
                        nc.sync.dma_start(
                            out_d.ap()[b, qc * P:(qc + 1) * P, :], o_sb[:])
            p2cm.__exit__(None, None, None)

    nc.compile()
    return nc


_NC_CACHE = []


def _get_nc():
    if not _NC_CACHE:
        _NC_CACHE.append(build_nc())
    return _NC_CACHE[0]


def make_in_maps(x, Wq, bq, Wk, bk, Wv, bv):
    in_maps = []
    for c in range(N_CORES):
        xt = np.ascontiguousarray(x[c].T)  # [D, S]
        xq = np.ascontiguousarray(
            x[:, c * QS:(c + 1) * QS, :].transpose(0, 2, 1))  # [B, D, QS]
        in_maps.append({
            "xt": xt, "xq": xq,
            "wq": Wq, "wk": Wk, "wv": Wv,
            "bq": bq, "bk": bk, "bv": bv,
            "onesv": np.ones((1, P), np.float32),
        })
    return in_maps


def kernel(x, Wq, bq, Wk, bk, Wv, bv):
    x = np.asarray(x, dtype=np.float32)
    Wq = np.asarray(Wq, dtype=np.float32)
    Wk = np.asarray(Wk, dtype=np.float32)
    Wv = np.asarray(Wv, dtype=np.float32)
    bq = np.asarray(bq, dtype=np.float32)
    bk = np.asarray(bk, dtype=np.float32)
    bv = np.asarray(bv, dtype=np.float32)

    nc = _get_nc()
    in_maps = make_in_maps(x, Wq, bq, Wk, bk, Wv, bv)
    res = bass_utils.run_bass_kernel_spmd(
        nc, in_maps, core_ids=list(range(N_CORES)))
    out = np.empty((B, S, E), np.float32)
    for c in range(N_CORES):
        out[:, c * QS:(c + 1) * QS, :] = res.results[c]["out"]
    return out


# revision 13
# speedup vs baseline: 2.5601x; 1.4941x over previous
"""Trainium2 Bass kernel for batch-axis-softmax attention.

Reference computation (B=8, S=2048, D_IN=512, D_OUT=256):
    q = relu(x @ Wq + bq); k = relu(x @ Wk + bk); v = relu(x @ Wv + bv)
    scores = q @ k^T / sqrt(256)            # [B, S, S]
    attn = softmax(scores, axis=0)          # softmax over the BATCH axis
    out = attn @ v                          # [B, S, D_OUT]

Sharding (8 cores): query-axis sharding. Core c owns query rows
[c*256, (c+1)*256) of every batch. Because the softmax runs over the batch
axis, each (q, k) position needs all 8 batches' scores — by giving every
core all batches for a slice of q-rows, the softmax is core-local.

Per core:
  phase 1: compute k^T and v (both bf16) for its own batch b=c from a
           host-pre-transposed x[c]^T.
  Two AllGathers (k first so the scores stage can start while v is still
  in flight), 1 MB/rank payload each.
  phase 2: project q for its 256 q-rows of all batches (from host-sliced
           x), scores^T = k_b @ q_slice^T (bf16 in, f32 accum), exp (no
           max subtraction needed: scores are in [0.18, 2.2]), Z = sum_b
           exp, attn = exp/Z, out = attn^T @ v.

Projections run in float32r (full PE rate at N>=256, near-fp32 accuracy);
the attention matmuls run bf16 (FWL weight loads). exp/attn/Z are bf16 —
the k-sum in attn @ v averages elementwise bf16 noise down by ~sqrt(2048).
"""

import numpy as np

import concourse.bacc as bacc
import concourse.mybir as mybir
import concourse.tile as tile
from concourse import bass_utils

F32 = mybir.dt.float32
F32R = mybir.dt.float32r
BF16 = mybir.dt.bfloat16

B = 8            # batch == number of cores
S = 2048         # sequence length
D = 512          # input dim
E = 256          # output dim
P = 128          # partitions
N_CORES = 8
QS = S // N_CORES  # q rows per core = 256

DC = D // P      # 4 d-chunks
EC = E // P      # 2 e-chunks
SC = S // P      # 16 s-chunks
SCALE = 1.0 / 16.0  # 1/sqrt(E)


def build_nc():
    nc = bacc.Bacc("TRN2", target_bir_lowering=False, debug=False,
                   num_devices=N_CORES)

    xt_d = nc.dram_tensor("xt", [D, S], F32, kind="ExternalInput")
    xq_d = nc.dram_tensor("xq", [B, D, QS], F32, kind="ExternalInput")
    wq_d = nc.dram_tensor("wq", [D, E], F32, kind="ExternalInput")
    wk_d = nc.dram_tensor("wk", [D, E], F32, kind="ExternalInput")
    wv_d = nc.dram_tensor("wv", [D, E], F32, kind="ExternalInput")
    bq_d = nc.dram_tensor("bq", [E], F32, kind="ExternalInput")
    bk_d = nc.dram_tensor("bk", [E], F32, kind="ExternalInput")
    bv_d = nc.dram_tensor("bv", [E], F32, kind="ExternalInput")
    ones_d = nc.dram_tensor("onesv", [1, P], F32, kind="ExternalInput")
    out_d = nc.dram_tensor("out", [B, QS, E], F32, kind="ExternalOutput")

    def mm(out, lhsT, rhs, start, stop):
        nc.tensor.matmul(out, lhsT, rhs, start=start, stop=stop)

    with tile.TileContext(nc) as tc:
        with tc.tile_pool(name="const", bufs=1) as cpool, \
             tc.tile_pool(name="dram", bufs=1, space="DRAM") as dram, \
             tc.tile_pool(name="stream", bufs=4) as stream, \
             tc.tile_pool(name="outp", bufs=4) as outpool:

            # k^T / v of own batch, bf16, flattened [128, *] viewed as f32
            cc_k = dram.tile([P, S], F32)
            cc_v = dram.tile([P, S], F32)
            cc_k_out = dram.tile([N_CORES * P, S], F32, addr_space="Shared")
            cc_v_out = dram.tile([N_CORES * P, S], F32, addr_space="Shared")

            # ---- constants ----
            wq_sb = cpool.tile([P, DC, E], F32R)
            wk_sb = cpool.tile([P, DC, E], F32R)
            wv_sb = cpool.tile([P, DC, E], F32R)
            nc.sync.dma_start(wq_sb[:], wq_d.ap().rearrange(
                "(dc p) e -> p dc e", p=P).bitcast(F32R))
            nc.sync.dma_start(wk_sb[:], wk_d.ap().rearrange(
                "(dc p) e -> p dc e", p=P).bitcast(F32R))
            nc.sync.dma_start(wv_sb[:], wv_d.ap().rearrange(
                "(dc p) e -> p dc e", p=P).bitcast(F32R))
            bq_sb = cpool.tile([P, EC], F32)
            bk_sb = cpool.tile([P, EC], F32)
            nc.sync.dma_start(bq_sb[:], bq_d.ap().rearrange(
                "(ec p) -> p ec", p=P))
            nc.sync.dma_start(bk_sb[:], bk_d.ap().rearrange(
                "(ec p) -> p ec", p=P))
            bv_row = cpool.tile([1, E], F32R)
            nc.sync.dma_start(bv_row[:], bv_d.ap().rearrange(
                "(a e) -> a e", a=1).bitcast(F32R))
            ones_row = cpool.tile([1, P], F32R)
            nc.sync.dma_start(ones_row[:], ones_d.ap().bitcast(F32R))

            # ---- phase 1: k^T and v for own batch ----
            with tc.tile_pool(name="p1", bufs=1) as p1pool, \
                 tc.tile_pool(name="p1ps", bufs=1, space="PSUM") as p1ps:
                xt_sb = p1pool.tile([P, DC, S], F32R)
                xt_r = xt_d.ap().rearrange(
                    "(dc p) s -> p dc s", p=P).bitcast(F32R)
                for dc in range(DC):
                    nc.sync.dma_start(xt_sb[:, dc, :], xt_r[:, dc, :])

                # k^T[e, s] = relu(Wk^T @ x^T + bk): lhsT = Wk chunk,
                # rhs = x^T chunk.  kt layout: [p, ec, s] bf16
                kt_sb = p1pool.tile([P, EC, S], BF16)
                for ec in range(EC):
                    for sh in range(2):  # halves of S
                        ps_k = p1ps.tile([P, 1024], F32, tag="kps", bufs=2)
                        for dc in range(DC):
                            for st in range(2):  # 512-wide moving tiles
                                mm(ps_k[:, st * 512:(st + 1) * 512],
                                   wk_sb[:, dc, ec * P:(ec + 1) * P],
                                   xt_sb[:, dc,
                                         sh * 1024 + st * 512:
                                         sh * 1024 + (st + 1) * 512],
                                   start=(dc == 0), stop=(dc == DC - 1))
                        nc.scalar.activation(
                            kt_sb[:, ec, sh * 1024:(sh + 1) * 1024],
                            ps_k[:],
                            mybir.ActivationFunctionType.Relu,
                            bias=bk_sb[:, ec:ec + 1])
                nc.sync.dma_start(cc_k[:].bitcast(BF16), kt_sb[:])

                # gather k early: scores can start as soon as this lands
                nc.gpsimd.collective_compute(
                    "AllGather",
                    mybir.AluOpType.bypass,
                    replica_groups=[list(range(N_CORES))],
                    ins=[cc_k[:]],
                    outs=[cc_k_out[:]],
                )

                # v[s, e] = relu(x @ Wv + bv): lhsT = x^T chunk (stationary),
                # rhs = Wv chunk; bias via rank-1 matmul ones^T @ bv_row
                v_sb = p1pool.tile([P, SC * E], BF16)
                for sp in range(SC // 2):
                    ps_v = p1ps.tile([P, 2 * E], F32, tag="vps", bufs=2)
                    for half in range(2):
                        st = sp * 2 + half
                        sl = ps_v[:, half * E:(half + 1) * E]
                        mm(sl, ones_row[0:1, :], bv_row[0:1, :],
                           start=True, stop=False)
                        for dc in range(DC):
                            mm(sl, xt_sb[:, dc, st * P:(st + 1) * P],
                               wv_sb[:, dc, :],
                               start=False, stop=(dc == DC - 1))
                    nc.scalar.activation(
                        v_sb[:, sp * 2 * E:(sp + 1) * 2 * E], ps_v[:],
                        mybir.ActivationFunctionType.Relu)
                nc.sync.dma_start(cc_v[:].bitcast(BF16), v_sb[:])

                nc.gpsimd.collective_compute(
                    "AllGather",
                    mybir.AluOpType.bypass,
                    replica_groups=[list(range(N_CORES))],
                    ins=[cc_v[:]],
                    outs=[cc_v_out[:]],
                )

            # ---- phase 2a: q projection for own q-slice, all batches ----
            p2cm = tc.tile_pool(name="p2", bufs=1)
            p2pool = p2cm.__enter__()
            qsl_sb = p2pool.tile([P, B, EC, QS], BF16)
            with tc.tile_pool(name="p2q", bufs=1) as qpool, \
                 tc.tile_pool(name="p2qps", bufs=1, space="PSUM") as qps:
                xq_sb = qpool.tile([P, B, DC, QS], F32R)
                nc.sync.dma_start(xq_sb[:], xq_d.ap().rearrange(
                    "b (dc p) s -> p b dc s", p=P).bitcast(F32R))
                for ec in range(EC):
                    ps_list = []
                    for b in range(B):
                        ps_q = qps.tile([P, QS], F32, tag="qps", bufs=B,
                                        name=f"ps_q_{ec}_{b}")
                        ps_list.append(ps_q)
                    for dc in range(DC):
                        for b in range(B):
                            mm(ps_list[b][:],
                               wq_sb[:, dc, ec * P:(ec + 1) * P],
                               xq_sb[:, b, dc, :],
                               start=(dc == 0), stop=(dc == DC - 1))
                    for b in range(B):
                        nc.scalar.activation(
                            qsl_sb[:, b, ec, :], ps_list[b][:],
                            mybir.ActivationFunctionType.Relu,
                            bias=bq_sb[:, ec:ec + 1])

            # ---- phase 2b: scores^T, exp, Z ----
            # exp_all[p, b, kc, q] = exp(scores[b, q, kc*128+p] / 16)
            exp_all = p2pool.tile([P, B, SC, QS], BF16)
            z_sb = p2pool.tile([P, SC, QS], BF16)
            r_sb = p2pool.tile([P, SC, QS], BF16)
            with tc.tile_pool(name="p2sps", bufs=1, space="PSUM") as sps:
                for b in range(B):
                    kt_s = stream.tile([P, 2 * S], BF16, tag="kv",
                                       name=f"kt_s{b}")
                    nc.sync.dma_start(
                        kt_s[:],
                        cc_k_out[b * P:(b + 1) * P, :].bitcast(BF16))
                    for half in range(2):
                        ps_s = sps.tile([P, 8, QS], F32, tag="sps", bufs=2,
                                        name=f"ps_s{b}_{half}")
                        for kc8 in range(8):
                            kc = half * 8 + kc8
                            for ec in range(EC):
                                mm(ps_s[:, kc8, :],
                                   kt_s[:, ec * S + kc * P:
                                        ec * S + (kc + 1) * P],
                                   qsl_sb[:, b, ec, :],
                                   start=(ec == 0), stop=(ec == EC - 1))
                        nc.scalar.activation(
                            exp_all[:, b, half * 8:(half + 1) * 8, :],
                            ps_s[:],
                            mybir.ActivationFunctionType.Exp,
                            scale=SCALE)
                    if b == 0:
                        nc.vector.tensor_copy(z_sb[:], exp_all[:, 0])
                    else:
                        nc.vector.tensor_add(z_sb[:], z_sb[:],
                                             exp_all[:, b])

            # ---- phase 2c: attn = exp * (1/Z), out = attn^T @ v ----
            # 1/Z = exp(-ln Z) on ScalarE: DVE RECIPROCAL on this tile
            # measures 25.8 us; two ACT passes cost ~7 us. The bf16 noise is
            # elementwise-independent across k positions; the 2048-term
            # k-sum in attn @ v averages it to ~1e-4.
            nc.scalar.activation(r_sb[:], z_sb[:],
                                 mybir.ActivationFunctionType.Ln)
            nc.scalar.activation(r_sb[:], r_sb[:],
                                 mybir.ActivationFunctionType.Exp,
                                 scale=-1.0)
            for b in range(B):
                eng = nc.vector if b % 2 == 0 else nc.gpsimd
                eng.tensor_mul(exp_all[:, b], exp_all[:, b], r_sb[:])

            with tc.tile_pool(name="p2ops", bufs=1, space="PSUM") as ops:
                for b in range(B):
                    v_s = stream.tile([P, SC * E], BF16, tag="kv",
                                      name=f"v_s{b}")
                    nc.sync.dma_start(
                        v_s[:],
                        cc_v_out[b * P:(b + 1) * P, :].bitcast(BF16))
                    ps_os = [ops.tile([P, E], F32, tag="ops", bufs=4,
                                      name=f"ps_o{b}_{qc}")
                             for qc in range(2)]
                    for st in range(SC):
                        for qc in range(2):
                            nc.tensor.matmul(
                                ps_os[qc][:],
                                exp_all[:, b, st, qc * P:(qc + 1) * P],
                                v_s[:, st * E:(st + 1) * E],
                                start=(st == 0), stop=(st == SC - 1))
                    for qc in range(2):
                        o_sb = outpool.tile([P, E], F32, tag="osb",
                                            name=f"o_sb{b}_{qc}")
                        nc.vector.tensor_copy(o_sb[:], ps_os[qc][:])
                        nc.sync.dma_start(
                            out_d.ap()[b, qc * P:(qc + 1) * P, :], o_sb[:])
            p2cm.__exit__(None, None, None)

    nc.compile()
    return nc


_NC_CACHE = []


def _get_nc():
    if not _NC_CACHE:
        _NC_CACHE.append(build_nc())
    return _NC_CACHE[0]


def make_in_maps(x, Wq, bq, Wk, bk, Wv, bv):
    in_maps = []
    for c in range(N_CORES):
        xt = np.ascontiguousarray(x[c].T)  # [D, S]
        xq = np.ascontiguousarray(
            x[:, c * QS:(c + 1) * QS, :].transpose(0, 2, 1))  # [B, D, QS]
        in_maps.append({
            "xt": xt, "xq": xq,
            "wq": Wq, "wk": Wk, "wv": Wv,
            "bq": bq, "bk": bk, "bv": bv,
            "onesv": np.ones((1, P), np.float32),
        })
    return in_maps


def kernel(x, Wq, bq, Wk, bk, Wv, bv):
    x = np.asarray(x, dtype=np.float32)
    Wq = np.asarray(Wq, dtype=np.float32)
    Wk = np.asarray(Wk, dtype=np.float32)
    Wv = np.asarray(Wv, dtype=np.float32)
    bq = np.asarray(bq, dtype=np.float32)
    bk = np.asarray(bk, dtype=np.float32)
    bv = np.asarray(bv, dtype=np.float32)

    nc = _get_nc()
    in_maps = make_in_maps(x, Wq, bq, Wk, bk, Wv, bv)
    res = bass_utils.run_bass_kernel_spmd(
        nc, in_maps, core_ids=list(range(N_CORES)))
    out = np.empty((B, S, E), np.float32)
    for c in range(N_CORES):
        out[:, c * QS:(c + 1) * QS, :] = res.results[c]["out"]
    return out
